# revision 15
# baseline (speedup 1.0000x reference)
"""Multi-head attention (RoPE, causal) Trainium2 kernel, 8-way sharded.

Sharding: core c => batch b = c//2, head-group g = c%2 (8 of 16 heads).
Each core computes Q/K/V projections for its (b, g), RoPE, causal
attention over its 8 heads, and the row-slice of the output projection.
Host sums the two partial output projections per batch and adds b_o.

Per-core dataflow (all fp32):
  - projections contract over model dim via PE; x supplied host-transposed
    [D, S] so both operands have the contraction on partitions.
  - Q/K projected to natural [s, d] tiles, RoPE applied with free-dim
    shifted DVE ops, then PE-transposed into QT/KT [d_headpair(128), S].
  - scoresT[k, q] per head via row-paired matmuls (two heads concurrently
    in row-groups 0-1 / 2-3 of the PE array; contraction = dk = 64).
  - softmax without max-subtraction (scores bounded ~|10| for this
    problem); exp on ACT straight out of PSUM; causal masking of diagonal
    blocks via gpsimd affine_select; fully-masked blocks skipped.
  - attn @ V with V' = [V | ones] as stationary (M=65): row 64 accumulates
    the softmax denominator for free. contextT stays unnormalized.
  - normalization: recip(den) broadcast across the 128 head-pair
    partitions with a K=2 indicator matmul, then fused into the PSUM->SBUF
    eviction muls.
  - output projection consumes contextT directly as lhsT (contraction =
    head dims on partitions); per-core result is a [S, D] partial sum.
"""

import json
import os

import numpy as np

# ---------------------------------------------------------------------------
# Workaround: this container's walrus accepts only ONE sync-wait per
# instruction. Hoist every instruction's waits onto single-wait NoOps
# inserted immediately before it (same engine, same program order).
# ---------------------------------------------------------------------------
_PATCHED = False


def _split_multiwait_bir(bir_json: bytes) -> bytes:
    m = json.loads(bir_json)
    ctr = 0
    changed = False
    for f in m.get("functions", []):
        for bl in f.get("blocks", []):
            out = []
            for inst in bl.get("instructions", []):
                si = inst.get("sync_info")
                ow = (si or {}).get("on_wait") or []
                if len(ow) > 1:
                    changed = True
                    for w in ow:
                        ctr += 1
                        out.append({
                            "debug": inst.get("debug", 0),
                            "engine": inst["engine"],
                            "ins": [],
                            "name": f"WSPLIT-{ctr}",
                            "opcode": "NoOp",
                            "outs": [],
                            "sync_info": {"on_update": [], "on_wait": [w]},
                        })
                    si["on_wait"] = []
                out.append(inst)
            if changed:
                bl["instructions"] = out
    if not changed:
        return bir_json
    return json.dumps(m).encode()


def _install_ntff_hook():
    """The agent image's antenv lacks the axon_hooks shim that bass_utils
    imports for trace=True under axon; synthesize it and register the
    ctypes-based NTFF hook from trn_agent_boot (degrades to no-trace if
    anything is missing)."""
    import sys
    import types

    if "antenv.axon_hooks" in sys.modules:
        return
    mod = types.ModuleType("antenv.axon_hooks")
    holder = [None]
    mod.set_axon_ntff_profile_hook = lambda h: holder.__setitem__(0, h)
    mod.get_axon_ntff_profile_hook = lambda: holder[0]
    sys.modules["antenv.axon_hooks"] = mod
    try:
        import antenv
        antenv.axon_hooks = mod
        from trn_agent_boot.trn_boot import _ntff_profile_via_ctypes
        mod.set_axon_ntff_profile_hook(
            _ntff_profile_via_ctypes("/opt/axon/libaxon_pjrt.so"))
    except Exception:
        pass


def _install_patches():
    global _PATCHED
    if _PATCHED:
        return
    import concourse.bass as bass

    orig = bass.Bass.to_json_bytes

    def to_json_bytes_patched(self, *a, **k):
        return _split_multiwait_bir(orig(self, *a, **k))

    bass.Bass.to_json_bytes = to_json_bytes_patched
    _install_ntff_hook()
    _PATCHED = True


# ---------------------------------------------------------------------------
# Problem constants (hardcoded per the harness contract)
# ---------------------------------------------------------------------------
B, S, D = 4, 2048, 1024
H, DK = 16, 64
HG = 8                    # heads per core
DG = HG * DK              # 512: head-group width
N_CORES = 8
ROPE_BASE = 10000.0
P = 128                   # partitions
ST = S // P               # 16 s-tiles
CC = D // P               # 8 contraction chunks for projections
QR = S // 512             # 4 q-ranges of 512
HPAIRS = HG // 2          # 4 head pairs
VSTRIDE = 65              # V columns + ones column


def _build_program(use_bias: bool, phases: int = 3):
    import concourse.bass as bass
    import concourse.mybir as mybir
    import concourse.tile as tile
    from concourse.masks import make_identity

    F32 = mybir.dt.float32
    nc = bass.Bass()

    xqT = nc.dram_tensor("xqT", [D, S], F32, kind="ExternalInput")
    xkT = nc.dram_tensor("xkT", [D, S], F32, kind="ExternalInput")
    xvT = nc.dram_tensor("xvT", [D, S], F32, kind="ExternalInput")
    wqT = nc.dram_tensor("wqT", [D, DG], F32, kind="ExternalInput")
    wkT = nc.dram_tensor("wkT", [D, DG], F32, kind="ExternalInput")
    wvT = nc.dram_tensor("wvT", [D, DG], F32, kind="ExternalInput")
    woT = nc.dram_tensor("woT", [DG, D], F32, kind="ExternalInput")
    cos_d = nc.dram_tensor("cos_d", [S, DK], F32, kind="ExternalInput")
    ssg_d = nc.dram_tensor("ssg_d", [S, DK], F32, kind="ExternalInput")
    if use_bias:
        bias_d = nc.dram_tensor("bias_d", [4, DG], F32, kind="ExternalInput")
        ones_d = nc.dram_tensor("ones_d", [1, P], F32, kind="ExternalInput")
    out_d = nc.dram_tensor("out", [S, D], F32, kind="ExternalOutput")

    with tile.TileContext(nc) as tc:
        with tc.tile_pool(name="consts", bufs=1) as consts, \
             tc.tile_pool(name="xT", bufs=8) as xT_pool, \
             tc.tile_pool(name="w", bufs=8) as w_pool, \
             tc.tile_pool(name="nat", bufs=3) as nat_pool, \
             tc.tile_pool(name="qk", bufs=8) as qk_pool, \
             tc.tile_pool(name="vp", bufs=1) as v_pool, \
             tc.tile_pool(name="ctx", bufs=4) as ctx_pool, \
             tc.tile_pool(name="den", bufs=1) as den_pool, \
             tc.tile_pool(name="w512", bufs=6) as work_pool, \
             tc.tile_pool(name="psm", bufs=6, space="PSUM") as ps_main, \
             tc.tile_pool(name="psc", bufs=2, space="PSUM") as ps_ctx:

            ident = consts.tile([P, P], F32)
            make_identity(nc, ident)
            ones1 = consts.tile([1, 64], F32)
            nc.vector.memset(ones1, 1.0)
            # cos/ssign: [S, 64] -> [128, 16*64] (s = st*128 + p)
            cos_sb = consts.tile([P, ST * DK], F32)
            nc.sync.dma_start(out=cos_sb,
                              in_=cos_d.rearrange("(t p) d -> p t d", p=P))
            ssg_sb = consts.tile([P, ST * DK], F32)
            nc.sync.dma_start(out=ssg_sb,
                              in_=ssg_d.rearrange("(t p) d -> p t d", p=P))
            if use_bias:
                bias_sb = consts.tile([4, DG], F32)
                nc.sync.dma_start(out=bias_sb, in_=bias_d[:, :])
                ones_sb = consts.tile([1, P], F32)
                nc.sync.dma_start(out=ones_sb, in_=ones_d[:, :])

            # persistent activations
            qT = [qk_pool.tile([P, S], F32, tag="qk", name=f"qT{i}") for i in range(HPAIRS)]
            kT = [qk_pool.tile([P, S], F32, tag="qk", name=f"kT{i}") for i in range(HPAIRS)]
            v_all = v_pool.tile([P, HG * ST * VSTRIDE], F32)
            # ones columns of V' (single strided memset)
            nc.gpsimd.memset(
                v_all.rearrange("p (h t c) -> p h t c", h=HG, t=ST)[:, :, :, DK:DK + 1],
                1.0)
            ctxT = [ctx_pool.tile([P, S], F32, tag="ctx", name=f"ctxT{i}") for i in range(HPAIRS)]

            # ---------------- projections + RoPE + transposes --------------
            def cos_bc(st, half):
                # cos/ssign slice [128, 32] broadcast over 8 heads
                src = cos_sb if half is None else ssg_sb
                width = DK if half is None else 32
                off = st * DK + (0 if half in (None, 0) else 32)
                sl = src[:, off:off + width]
                return bass.AP(tensor=sl.tensor, offset=sl.offset,
                               ap=[sl.ap[0], [0, HG], [1, width]])

            for t_i, (x_t, w_t) in enumerate(((xqT, wqT), (xkT, wkT), (xvT, wvT))):
                for sg in range(QR):           # groups of 4 s-tiles
                    xg = [xT_pool.tile([P, 512], F32, tag="xT", name=f"xg{i}") for i in range(CC)]
                    for cc in range(CC):
                        nc.sync.dma_start(
                            out=xg[cc],
                            in_=x_t[cc * P:(cc + 1) * P, sg * 512:(sg + 1) * 512])
                    if sg == 0:
                        wg = [w_pool.tile([P, DG], F32, tag="w", name=f"wg{i}") for i in range(CC)]
                        for cc in range(CC):
                            nc.sync.dma_start(
                                out=wg[cc], in_=w_t[cc * P:(cc + 1) * P, :])
                    for sti in range(4):
                        st = sg * 4 + sti
                        psum = ps_main.tile([P, DG], F32, tag="ps")
                        if use_bias:
                            nc.tensor.matmul(psum, ones_sb,
                                             bias_sb[t_i:t_i + 1, :],
                                             start=True, stop=False)
                        for cc in range(CC):
                            nc.tensor.matmul(
                                psum, xg[cc][:, sti * P:(sti + 1) * P], wg[cc],
                                start=(cc == 0 and not use_bias),
                                stop=(cc == CC - 1))
                        if t_i < 2:
                            # RoPE: nat = psum*cos ; nat += shift(psum)*ssign
                            nat = nat_pool.tile([P, DG], F32, tag="nat")
                            tmp = work_pool.tile([P, DG], F32, tag="w512")
                            nat4 = nat.rearrange("p (h t d) -> p h t d", h=HG, t=2)
                            tmp4 = tmp.rearrange("p (h t d) -> p h t d", h=HG, t=2)
                            ps4 = psum.rearrange("p (h t d) -> p h t d", h=HG, t=2)
                            nc.vector.tensor_mul(
                                nat.rearrange("p (h d) -> p h d", h=HG),
                                psum.rearrange("p (h d) -> p h d", h=HG),
                                cos_bc(st, None))
                            nc.vector.tensor_mul(tmp4[:, :, 0, :], ps4[:, :, 1, :],
                                                 cos_bc(st, 0))
                            nc.vector.tensor_mul(tmp4[:, :, 1, :], ps4[:, :, 0, :],
                                                 cos_bc(st, 1))
                            nc.vector.tensor_add(nat, nat, tmp)
                            dest = qT if t_i == 0 else kT
                            for hp in range(HPAIRS):
                                pt = ps_main.tile([P, P], F32, tag="ps")
                                nc.tensor.transpose(
                                    pt, nat[:, hp * P:(hp + 1) * P], ident)
                                nc.vector.tensor_copy(
                                    dest[hp][:, st * P:(st + 1) * P], pt)
                        else:
                            v4 = v_all.rearrange("p (h t c) -> p h t c",
                                                 h=HG, t=ST)
                            for h in range(HG):
                                nc.vector.tensor_copy(
                                    v4[:, h, st, 0:DK],
                                    psum[:, h * DK:(h + 1) * DK])

            if phases < 2:
                for i in range(4):
                    ot = work_pool.tile([P, 512], F32, tag="w512",
                                        name=f"dump{i}")
                    nc.vector.tensor_copy(ot, qT[i][:, 0:512])
                    nc.sync.dma_start(out=out_d[i * P:(i + 1) * P, 0:512], in_=ot)
                return nc
            # ----------------------- attention -----------------------------
            is_ge = mybir.AluOpType.is_ge
            Exp = mybir.ActivationFunctionType.Exp
            for hp in range(HPAIRS):
                hA, hB = 2 * hp, 2 * hp + 1
                for qr in range(QR):
                    pcA = ps_ctx.tile([VSTRIDE, 512], F32, tag="pc")
                    pcB = ps_ctx.tile([VSTRIDE, 512], F32, tag="pc")
                    n_kc = 4 * (qr + 1)
                    for kc in range(n_kc):
                        psA = ps_main.tile([P, 512], F32, tag="ps")
                        psB = ps_main.tile([P, 512], F32, tag="ps")
                        qsl = slice(qr * 512, (qr + 1) * 512)
                        ksl = slice(kc * P, (kc + 1) * P)
                        nc.tensor.matmul(psA, kT[hp][0:64, ksl], qT[hp][0:64, qsl],
                                         start=True, stop=True, tile_position=(0, 0))
                        nc.tensor.matmul(psB, kT[hp][64:128, ksl], qT[hp][64:128, qsl],
                                         start=True, stop=True, tile_position=(64, 0))
                        eA = work_pool.tile([P, 512], F32, tag="w512")
                        eB = work_pool.tile([P, 512], F32, tag="w512")
                        nc.scalar.activation(out=eA, in_=psA, func=Exp, scale=0.125)
                        nc.scalar.activation(out=eB, in_=psB, func=Exp, scale=0.125)
                        j = kc - 4 * qr
                        if j >= 0:  # diagonal block: keep qq - kk - 128*j >= 0
                            for e in (eA, eB):
                                nc.gpsimd.affine_select(
                                    out=e, in_=e, compare_op=is_ge, fill=0.0,
                                    base=-128 * j, channel_multiplier=-1,
                                    pattern=[[1, 512]])
                        v4 = v_all.rearrange("p (h t c) -> p h t c", h=HG, t=ST)
                        nc.tensor.matmul(pcA, v4[:, hA, kc, :], eA,
                                         start=(kc == 0), stop=(kc == n_kc - 1))
                        nc.tensor.matmul(pcB, v4[:, hB, kc, :], eB,
                                         start=(kc == 0), stop=(kc == n_kc - 1))
                    qsl = slice(qr * 512, (qr + 1) * 512)
                    recA = den_pool.tile([1, 512], F32, tag="rec", bufs=4,
                                         name="recA")
                    recB = den_pool.tile([1, 512], F32, tag="rec", bufs=4,
                                         name="recB")
                    nc.vector.reciprocal(out=recA, in_=pcA[64:65, :])
                    nc.vector.reciprocal(out=recB, in_=pcB[64:65, :])
                    pbc = ps_main.tile([P, 512], F32, tag="ps")
                    nc.tensor.matmul(pbc[0:64, :], ones1, recA,
                                     start=True, stop=True, tile_position=(0, 0),
                                     skip_group_check=True)
                    nc.tensor.matmul(pbc[64:128, :], ones1, recB,
                                     start=True, stop=True, tile_position=(0, 64),
                                     skip_group_check=True)
                    rbc = work_pool.tile([P, 512], F32, tag="w512")
                    nc.scalar.copy(rbc, pbc)
                    nc.vector.tensor_mul(ctxT[hp][0:64, qsl], pcA[0:64, :],
                                         rbc[0:64, :])
                    nc.vector.tensor_mul(ctxT[hp][64:128, qsl], pcB[0:64, :],
                                         rbc[64:128, :])

            if phases < 3:
                for i in range(4):
                    ot = work_pool.tile([P, 512], F32, tag="w512",
                                        name=f"dump{i}")
                    nc.vector.tensor_copy(ot, ctxT[i][:, 0:512])
                    nc.sync.dma_start(out=out_d[i * P:(i + 1) * P, 0:512], in_=ot)
                return nc
            # -------------------- output projection ------------------------
            for nr in range(2):
                wo = [w_pool.tile([P, 512], F32, tag="w", name=f"wo{nr}_{i}")
                      for i in range(4)]
                for dc in range(4):
                    nc.sync.dma_start(
                        out=wo[dc],
                        in_=woT[dc * P:(dc + 1) * P, nr * 512:(nr + 1) * 512])
                for st in range(ST):
                    po = ps_main.tile([P, 512], F32, tag="ps")
                    for dc in range(4):
                        nc.tensor.matmul(
                            po, ctxT[dc][:, st * P:(st + 1) * P], wo[dc],
                            start=(dc == 0), stop=(dc == 3))
                    ot = work_pool.tile([P, 512], F32, tag="w512")
                    nc.vector.tensor_copy(ot, po)
                    nc.sync.dma_start(
                        out=out_d[st * P:(st + 1) * P, nr * 512:(nr + 1) * 512],
                        in_=ot)
    return nc


_PROG_CACHE = {}


def _get_program(use_bias: bool):
    if use_bias not in _PROG_CACHE:
        _PROG_CACHE[use_bias] = _build_program(use_bias)
    return _PROG_CACHE[use_bias]


def _rope_tables():
    inv = 1.0 / (ROPE_BASE ** (np.arange(0, DK, 2, dtype=np.float32) / DK))
    t = np.arange(S, dtype=np.float32)
    fr = t[:, None] * inv[None, :]                      # [S, 32]
    emb = np.concatenate([fr, fr], axis=-1)             # [S, 64]
    cos = np.cos(emb).astype(np.float32)
    sin = np.sin(emb).astype(np.float32)
    ssg = sin.copy()
    ssg[:, :32] = -sin[:, :32]
    return cos, ssg


def kernel(query, key, value, W_q, b_q, W_k, b_k, W_v, b_v, W_o, b_o):
    _install_patches()
    from concourse.bass_utils import run_bass_kernel_spmd

    query = np.asarray(query, dtype=np.float32)
    key = np.asarray(key, dtype=np.float32)
    value = np.asarray(value, dtype=np.float32)
    W_q, W_k, W_v, W_o = (np.asarray(w, dtype=np.float32)
                          for w in (W_q, W_k, W_v, W_o))
    b_q, b_k, b_v, b_o = (np.asarray(b, dtype=np.float32)
                          for b in (b_q, b_k, b_v, b_o))

    use_bias = bool(np.any(b_q) or np.any(b_k) or np.any(b_v))
    nc = _get_program(use_bias)

    cos, ssg = _rope_tables()

    in_maps = []
    for c in range(N_CORES):
        b, g = divmod(c, 2)
        gs = slice(g * DG, (g + 1) * DG)
        m = {
            "xqT": np.ascontiguousarray(query[b].T),
            "xkT": np.ascontiguousarray(key[b].T),
            "xvT": np.ascontiguousarray(value[b].T),
            "wqT": np.ascontiguousarray(W_q[gs, :].T),
            "wkT": np.ascontiguousarray(W_k[gs, :].T),
            "wvT": np.ascontiguousarray(W_v[gs, :].T),
            "woT": np.ascontiguousarray(W_o[:, gs].T),
            "cos_d": cos,
            "ssg_d": ssg,
        }
        if use_bias:
            m["bias_d"] = np.stack([b_q[gs], b_k[gs], b_v[gs],
                                    np.zeros(DG, np.float32)])
            m["ones_d"] = np.ones((1, P), np.float32)
        in_maps.append(m)

    trace = bool(int(os.environ.get("KERNEL_TRACE", "0")))
    trace_cores = None
    if trace:
        tc_env = os.environ.get("KERNEL_TRACE_CORES", "")
        trace_cores = ([int(x) for x in tc_env.split(",") if x != ""]
                       if tc_env else list(range(N_CORES)))
    try:
        res = run_bass_kernel_spmd(nc, in_maps, core_ids=list(range(N_CORES)),
                                   trace=trace, trace_cores=trace_cores)
    except Exception:
        if not trace:
            raise
        res = run_bass_kernel_spmd(nc, in_maps, core_ids=list(range(N_CORES)),
                                   trace=False)
    kernel._last_results = res

    out = np.empty((B, S, D), np.float32)
    for b in range(B):
        out[b] = res.results[2 * b]["out"] + res.results[2 * b + 1]["out"] + b_o
    return out
